# revision 19
# baseline (speedup 1.0000x reference)
"""CausalTemporalAttnBlock Trainium2 kernel.

Problem: out = x + Wp @ attn(norm(x)) + bp, where norm is GroupNorm(1 group)
over (c,t,h,w) per batch, attention is causal over t, independent per (b,h,w).
Shapes: x (2, 512, 64, 32, 32) fp32; four (512,512) weights + biases.

Strategy (8 NeuronCores, ZERO cross-core communication):
  - core i handles batch i//4, h-rows [8*(i%4), 8*(i%4)+8), all w: 256 (h,w)
    locations per core.
  - The GroupNorm stats (mean/var per batch, 4 floats total) are computed on
    the host and folded into the projection weights, so the device kernel is a
    single pass over x with no collective and no stats prepass:
        q = Aq @ x + dq,  Aq = s*r*(Wq diag(gamma)),
        dq = s*(bq + Wq@beta - mu*r*Wq@gamma),  s = 1/sqrt(c), r = rstd.
    The K bias shifts every softmax row by a per-row constant -> dropped
    (exact). The V bias adds a per-channel constant to every attention output
    (softmax rows sum to 1) -> folded into the P bias: bp_eff = bp + Wp@dv
    (exact). Only the Q bias survives; it is applied at PSUM-eviction time.
  - Host re-lays the shard as [8 h-rows][512 c][32 w * 64 t] (w-major!) so
    every DMA is >=8KB contiguous AND every matmul operand -- the per-group
    x slab, the per-location x column block, and the residual slab -- is a
    plain contiguous slice. (A t-major layout makes the Q/K moving operand a
    64B-strided gather, which measured ~5x slower per matmul and kept the PE
    HAM-throttled at half clock.)
  - All matmuls run in bf16 (fp32 streams the moving operand at half rate --
    2 cycles/column -- so bf16 doubles PE throughput; fp32 accumulation in
    PSUM throughout, and the residual add stays full fp32). x is cast to bf16
    once per block on the otherwise-idle GpSimd engine; every other cast is
    folded into an eviction that already existed.
  - Per h-row block (32 locs), per group of 8 locs: Q/K projections
    (c-on-partitions), V produced transposed (VT, s-on-partitions) directly
    by making x the stationary operand, scores computed transposed
    S^T = K^T Q (s-on-partitions) so softmax normalization is a ones-matmul
    and AV needs no transposes at all. No max-subtraction (scores are O(1);
    exp is safe). Causal mask applied as a 0/1 multiply after exp.
  - Adjacent location PAIRS are packed into single full-array matmuls:
      * VT for a pair is ONE [128,128]x[128,512] matmul (the two locations'
        x columns are contiguous in the w-major layout); rows 0-63 hold the
        even location's V^T, rows 64-127 the odd one's.
      * Pair scores are ONE [128,128]x[128,128] matmul K_pair^T Q_pair whose
        off-diagonal blocks are cross-location garbage -- the causal mask is
        extended to a block-diagonal mask that zeroes them after exp. With
        the garbage zeroed, softmax row-sums are plain full-column sums and
        AV is ONE [128,128]x[128,128] matmul per (pair, c-chunk) contracting
        over all 128 packed s rows (the zeroed entries contribute nothing).
    This cuts the per-group matmul count from 146 to 98 and makes every
    attention matmul use the full 128-partition array.
  - Multi-chain PSUM banks (scores, AV) rely on start=True of the bank's
    first matmul to clear the whole bank; later chains overwrite where the
    has_written bit is unset -- no memsets.
"""

import ml_dtypes
import numpy as np

import concourse.bass as bass
import concourse.tile as tile
from concourse import bacc, mybir
from concourse.bass_utils import run_bass_kernel_spmd

P = 128
B, C, T, H, W = 2, 512, 64, 32, 32
NCORES = 8
HSH = H // 4          # 8 h-rows per core
CCH = C // P          # 4 c chunks
GRP = 8               # locations per attention group
NGRP = W // GRP       # 4 groups per block
EPS = 1e-6

f32 = mybir.dt.float32
f32r = mybir.dt.float32r
bf16 = mybir.dt.bfloat16
AX = mybir.AxisListType.X
ALU = mybir.AluOpType
AF = mybir.ActivationFunctionType


def build_nc(num_cores=NCORES, nblk=HSH):
    nc = bacc.Bacc("TRN2", target_bir_lowering=False, debug=False,
                   num_devices=num_cores)

    xs = nc.declare_dram_parameter("xs", [nblk, C, T * W], f32r, isOutput=False)
    wts = {}
    for nm in ("q", "k", "v", "p"):
        wts[nm] = nc.declare_dram_parameter(f"w{nm}t", [C, C], bf16,
                                            isOutput=False)
    dqcol = nc.declare_dram_parameter("dqcol", [P, CCH], f32, isOutput=False)
    bpcol = nc.declare_dram_parameter("bpcol", [P, CCH], f32, isOutput=False)
    maskp = nc.declare_dram_parameter("maskt", [P, 4 * P], bf16, isOutput=False)
    ones_col = nc.declare_dram_parameter("ones_col", [P, 1], bf16, isOutput=False)
    ones_row = nc.declare_dram_parameter("ones_row", [1, C], bf16, isOutput=False)
    outp = nc.declare_dram_parameter("out", [nblk, C, T * W], f32,
                                     isOutput=True)

    with tile.TileContext(nc) as tc:
        with (
            tc.tile_pool(name="const", bufs=1) as const,
            tc.tile_pool(name="xpool", bufs=2) as xpool,
            tc.tile_pool(name="cpool", bufs=2) as cpool,
            tc.tile_pool(name="gpool", bufs=2) as gpool,
            tc.tile_pool(name="spool", bufs=2) as spool,
            tc.tile_pool(name="pp", bufs=4, space="PSUM") as pp,
            tc.tile_pool(name="pss", bufs=2, space="PSUM") as pss,
            tc.tile_pool(name="psm", bufs=1, space="PSUM") as psm,
        ):
            # ---------- constants ----------
            w_sb = {}
            for nm in ("q", "k", "v", "p"):
                for ci in range(CCH):
                    t = const.tile([P, C], bf16, tag=f"w{nm}{ci}")
                    nc.sync.dma_start(t[:], wts[nm][ci * P:(ci + 1) * P, :])
                    w_sb[nm, ci] = t
            dq_sb = const.tile([P, CCH], f32, tag="dqcol")
            nc.sync.dma_start(dq_sb[:], dqcol[:])
            bp_sb = const.tile([P, CCH], f32, tag="bpcol")
            nc.sync.dma_start(bp_sb[:], bpcol[:])
            mask_sb = const.tile([P, 4 * P], bf16, tag="maskt")
            nc.sync.dma_start(mask_sb[:], maskp[:])
            ocr_sb = const.tile([P, 1], bf16, tag="ocr")
            nc.sync.dma_start(ocr_sb[:], ones_col[:])
            orr_sb = const.tile([1, C], bf16, tag="orr")
            nc.sync.dma_start(orr_sb[:], ones_row[:])

            # ---------- main blocks ----------
            for blk in range(nblk):
                xb = []
                xc = []
                for ci in range(CCH):
                    t = xpool.tile([P, T * W], f32r, tag=f"xb{ci}")
                    nc.sync.dma_start(t[:], xs[blk, ci * P:(ci + 1) * P, :])
                    xb.append(t)
                    c = cpool.tile([P, T * W], bf16, tag=f"xc{ci}")
                    nc.gpsimd.tensor_copy(c[:], t[:].bitcast(f32))
                    xc.append(c)

                def xgrp(ci, w0, n=GRP):
                    # [128, n*64] contiguous bf16 slab of group cols (w-major)
                    return xc[ci][:, w0 * T:(w0 + n) * T]

                def xres(ci, w0, n=GRP):
                    # fp32 residual slab
                    return xb[ci][:, w0 * T:(w0 + n) * T]



                for g in range(NGRP):
                    w0 = g * GRP
                    # ---- Q, K projections: psum[co, (w,t)] over ci ----
                    qk = {}
                    for nm in ("q", "k"):
                        for co in range(CCH):
                            ps = pp.tile([P, 512], f32, tag="pp")
                            for ci in range(CCH):
                                nc.tensor.matmul(
                                    ps[:], w_sb[nm, ci][:, co * P:(co + 1) * P],
                                    xgrp(ci, w0), start=(ci == 0),
                                    stop=(ci == CCH - 1))
                            t = gpool.tile([P, 512], bf16, tag=f"{nm}g{co}")
                            if nm == "q":
                                # q += dq at eviction (DVE), cast to bf16
                                nc.vector.tensor_scalar(
                                    t[:], ps[:], dq_sb[:, co:co + 1], None,
                                    ALU.add)
                            else:
                                nc.scalar.copy(t[:], ps[:])
                            qk[nm, co] = t

                    # ---- VT pairs: [128 packed s, 512 co] per pair ----
                    # lhsT = two adjacent locations' x columns [128,128];
                    # rows 0-63 of the result hold the even location's V^T,
                    # rows 64-127 the odd one's
                    vt = []
                    for j in range(GRP // 2):
                        ps = pss.tile([P, 512], f32, tag="ppv")
                        for ci in range(CCH):
                            nc.tensor.matmul(ps[:], xgrp(ci, w0 + 2 * j, 2),
                                             w_sb["v", ci][:],
                                             start=(ci == 0),
                                             stop=(ci == CCH - 1))
                        t = gpool.tile([P, 512], bf16, tag=f"vtg{j}")
                        nc.scalar.copy(t[:], ps[:])
                        vt.append(t)

                    # ---- pair scores: S2[packed s, (pair, t)] ----
                    # one [128,128]x[128,128] matmul per (pair, ci); the
                    # off-diagonal cross-location blocks are garbage and get
                    # zeroed by the block-diagonal causal mask after exp.
                    # one bank holds 4 chains; the first matmul's start=True
                    # clears the whole bank, later chains overwrite where
                    # has_written is unset
                    ps_s = psm.tile([P, 512], f32, tag="pss")
                    for j in range(GRP // 2):
                        for ci in range(CCH):
                            kl = qk["k", ci][:, j * P:(j + 1) * P]
                            ql = qk["q", ci][:, j * P:(j + 1) * P]
                            nc.tensor.matmul(ps_s[:, j * P:(j + 1) * P],
                                             kl, ql,
                                             start=(j == 0 and ci == 0),
                                             stop=(ci == CCH - 1),
                                             skip_group_check=True)
                    # ---- softmax (no max-subtraction) ----
                    pexp = spool.tile([P, 512], bf16, tag="pexp")
                    nc.scalar.activation(pexp[:], ps_s[:], AF.Exp)
                    pm = spool.tile([P, 512], bf16, tag="pmask")
                    nc.vector.tensor_mul(pm[:], pexp[:], mask_sb[:])
                    # masked rows are zero, so full-column sums are exactly
                    # the per-location softmax denominators. The sum row and
                    # its broadcast share one PSUM bank (the sum is consumed
                    # by the reciprocal before the broadcast overwrites it).
                    pmix = psm.tile([P, 512], f32, tag="pmix")
                    nc.tensor.matmul(pmix[0:1, :], ocr_sb[:], pm[:],
                                     start=True, stop=True)
                    rsf = spool.tile([1, 512], f32, tag="rsf")
                    nc.vector.reciprocal_approx_fast(rsf[:], pmix[0:1, :])
                    rs = spool.tile([1, 512], bf16, tag="rs")
                    nc.vector.tensor_copy(rs[:], rsf[:])
                    nc.tensor.matmul(pmix[:], orr_sb[0:1, 0:P], rs[:],
                                     start=True, stop=True)
                    pn = spool.tile([P, 512], bf16, tag="pn")
                    nc.vector.tensor_mul(pn[:], pm[:], pmix[:])

                    # ---- AV: O[c,(w,t)], one matmul per (pair, c-chunk)
                    # contracting over all 128 packed s rows (zeroed entries
                    # of pn contribute nothing) ----
                    og = []
                    for ch in range(CCH):
                        ps_o = pss.tile([P, 512], f32, tag="ppv")
                        for j in range(GRP // 2):
                            lhsT = vt[j][:, ch * P:(ch + 1) * P]
                            nc.tensor.matmul(ps_o[:, j * P:(j + 1) * P],
                                             lhsT, pn[:, j * P:(j + 1) * P],
                                             start=(j == 0), stop=True,
                                             skip_group_check=True)
                        t = gpool.tile([P, 512], bf16, tag=f"og{ch}")
                        nc.scalar.copy(t[:], ps_o[:])
                        og.append(t)

                    # ---- P-projection + bias + residual ----
                    for co in range(CCH):
                        ps = pp.tile([P, 512], f32, tag="pp")
                        for ci in range(CCH):
                            nc.tensor.matmul(
                                ps[:], w_sb["p", ci][:, co * P:(co + 1) * P],
                                og[ci][:], start=(ci == 0),
                                stop=(ci == CCH - 1))
                        # x += (ps + bp_eff): fused bias + residual on DVE
                        xsl = xres(co, w0)
                        nc.vector.scalar_tensor_tensor(
                            xsl, ps[:], bp_sb[:, co:co + 1], xsl.bitcast(f32),
                            ALU.add, ALU.add)

                for ci in range(CCH):
                    nc.sync.dma_start(outp[blk, ci * P:(ci + 1) * P, :],
                                      xb[ci][:].bitcast(f32))
    nc.compile()
    return nc


def host_prep(x, gamma, beta, wq, bq, wk, bk, wv, bv, wp, bp):
    """Per-batch GroupNorm stats + fold gamma/beta/mean/rstd into weights.

    Returns a list of per-batch constant dicts (cores 0-3 use batch 0,
    cores 4-7 use batch 1)."""
    n = C * T * H * W
    s = np.float32(1.0 / np.sqrt(C))
    g64 = gamma.astype(np.float64)
    b64 = beta.astype(np.float64)

    # block-diagonal causal mask for packed location pairs: [[M,0],[0,M]]
    # per pair (M = triu), tiled across the 4 pairs of a group
    m1 = np.triu(np.ones((T, T), np.float32))
    z = np.zeros((T, T), np.float32)
    mblk = np.block([[m1, z], [z, m1]])                       # (128, 128)
    maskt = np.tile(mblk, (1, 4))                             # (128, 512)
    shared = {
        "maskt": maskt.astype(ml_dtypes.bfloat16),
        "ones_col": np.ones((P, 1), ml_dtypes.bfloat16),
        "ones_row": np.ones((1, C), ml_dtypes.bfloat16),
    }

    out = []
    for b in range(B):
        y = x[b].reshape(-1)
        s1 = float(np.add.reduce(y, dtype=np.float64))
        s2 = float(np.add.reduce(np.square(y, dtype=np.float64)))
        mu = s1 / n
        var = s2 / n - mu * mu
        r = 1.0 / np.sqrt(var + EPS)

        def fold(w, bias, scale):
            w64 = w.astype(np.float64)
            a = (w64 * g64[None, :]) * (scale * r)            # (co, ci)
            d = (bias.astype(np.float64) + w64 @ b64
                 - (mu * r) * (w64 @ g64)) * scale            # (co,)
            return np.ascontiguousarray(a.T.astype(ml_dtypes.bfloat16)), d

        aqt, dq = fold(wq, bq, s)
        akt, _ = fold(wk, bk, 1.0)
        avt, dv = fold(wv, bv, 1.0)
        apt = np.ascontiguousarray(wp.T.astype(ml_dtypes.bfloat16))
        bp_eff = bp.astype(np.float64) + wp.astype(np.float64) @ dv

        dqcol = np.empty((P, CCH), np.float32)
        bpcol = np.empty((P, CCH), np.float32)
        for ch in range(CCH):
            dqcol[:, ch] = dq[ch * P:(ch + 1) * P]
            bpcol[:, ch] = bp_eff[ch * P:(ch + 1) * P]

        out.append({
            "wqt": aqt, "wkt": akt, "wvt": avt, "wpt": apt,
            "dqcol": dqcol, "bpcol": bpcol, **shared,
        })
    return out


_NC_CACHE = {}


def kernel(x, gamma, beta, wq, bq, wk, bk, wv, bv, wp, bp):
    x = np.asarray(x, np.float32)
    args = [np.asarray(a, np.float32) for a in
            (gamma, beta, wq, bq, wk, bk, wv, bv, wp, bp)]
    consts = host_prep(x, *args)

    if "nc" not in _NC_CACHE:
        _NC_CACHE["nc"] = build_nc()
    nc = _NC_CACHE["nc"]

    in_maps = []
    for core in range(NCORES):
        b, hg = core // 4, core % 4
        shard = x[b, :, :, hg * HSH:(hg + 1) * HSH, :]        # (C,T,HSH,W)
        shard = np.ascontiguousarray(
            shard.transpose(2, 0, 3, 1)).reshape(HSH, C, W * T)
        in_maps.append({"xs": shard, **consts[b]})

    global _last_in_maps
    _last_in_maps = in_maps
    res = run_bass_kernel_spmd(nc, in_maps, list(range(NCORES)))

    out = np.empty((B, C, T, H, W), np.float32)
    for core in range(NCORES):
        b, hg = core // 4, core % 4
        o = res.results[core]["out"].reshape(HSH, C, W, T)
        out[b, :, :, hg * HSH:(hg + 1) * HSH, :] = o.transpose(1, 3, 0, 2)
    return out


# revision 21
# speedup vs baseline: 1.1648x; 1.1648x over previous
"""CausalTemporalAttnBlock Trainium2 kernel.

Problem: out = x + Wp @ attn(norm(x)) + bp, where norm is GroupNorm(1 group)
over (c,t,h,w) per batch, attention is causal over t, independent per (b,h,w).
Shapes: x (2, 512, 64, 32, 32) fp32; four (512,512) weights + biases.

Strategy (8 NeuronCores, ZERO cross-core communication):
  - core i handles batch i//4, h-rows [8*(i%4), 8*(i%4)+8), all w: 256 (h,w)
    locations per core.
  - The GroupNorm stats (mean/var per batch, 4 floats total) are computed on
    the host and folded into the projection weights, so the device kernel is a
    single pass over x with no collective and no stats prepass:
        q = Aq @ x + dq,  Aq = s*r*(Wq diag(gamma)),
        dq = s*(bq + Wq@beta - mu*r*Wq@gamma),  s = 1/sqrt(c), r = rstd.
    The K bias shifts every softmax row by a per-row constant -> dropped
    (exact). The V bias adds a per-channel constant to every attention output
    (softmax rows sum to 1) -> folded into the P bias: bp_eff = bp + Wp@dv
    (exact). Only the Q bias survives; it is applied at PSUM-eviction time.
  - Host re-lays the shard as [8 h-rows][512 c][32 w * 64 t] (w-major!) so
    every DMA is >=8KB contiguous AND every matmul operand -- the per-group
    x slab, the per-location x column block, and the residual slab -- is a
    plain contiguous slice. (A t-major layout makes the Q/K moving operand a
    64B-strided gather, which measured ~5x slower per matmul and kept the PE
    HAM-throttled at half clock.)
  - All matmuls run in bf16 (fp32 streams the moving operand at half rate --
    2 cycles/column -- so bf16 doubles PE throughput; fp32 accumulation in
    PSUM throughout, and the residual add stays full fp32). x is cast to bf16
    once per block on the otherwise-idle GpSimd engine; every other cast is
    folded into an eviction that already existed.
  - Per h-row block (32 locs), per group of 8 locs: Q/K projections
    (c-on-partitions), V produced transposed (VT, s-on-partitions) directly
    by making x the stationary operand, scores computed transposed
    S^T = K^T Q (s-on-partitions) so softmax normalization is a ones-matmul
    and AV needs no transposes at all. No max-subtraction (scores are O(1);
    exp is safe). Causal mask applied as a 0/1 multiply after exp.
  - Adjacent location PAIRS are packed into single full-array matmuls:
      * VT for a pair is ONE [128,128]x[128,512] matmul (the two locations'
        x columns are contiguous in the w-major layout); rows 0-63 hold the
        even location's V^T, rows 64-127 the odd one's.
      * Pair scores are ONE [128,128]x[128,128] matmul K_pair^T Q_pair whose
        off-diagonal blocks are cross-location garbage -- the causal mask is
        extended to a block-diagonal mask that zeroes them after exp. With
        the garbage zeroed, softmax row-sums are plain full-column sums and
        AV is ONE [128,128]x[128,128] matmul per (pair, c-chunk) contracting
        over all 128 packed s rows (the zeroed entries contribute nothing).
    This cuts the per-group matmul count from 146 to 98 and makes every
    attention matmul use the full 128-partition array.
  - Multi-chain PSUM banks (scores, AV) rely on start=True of the bank's
    first matmul to clear the whole bank; later chains overwrite where the
    has_written bit is unset -- no memsets.
"""

import ml_dtypes
import numpy as np

import concourse.bass as bass
import concourse.tile as tile
from concourse import bacc, mybir
from concourse.bass_utils import run_bass_kernel_spmd

P = 128
B, C, T, H, W = 2, 512, 64, 32, 32
NCORES = 8
HSH = H // 4          # 8 h-rows per core
CCH = C // P          # 4 c chunks
GRP = 8               # locations per attention group
NGRP = W // GRP       # 4 groups per block
EPS = 1e-6

f32 = mybir.dt.float32
f32r = mybir.dt.float32r
bf16 = mybir.dt.bfloat16
AX = mybir.AxisListType.X
ALU = mybir.AluOpType
AF = mybir.ActivationFunctionType


def build_nc(num_cores=NCORES, nblk=HSH):
    nc = bacc.Bacc("TRN2", target_bir_lowering=False, debug=False,
                   num_devices=num_cores)

    xs = nc.declare_dram_parameter("xs", [nblk, C, T * W], f32r, isOutput=False)
    wts = {}
    for nm in ("q", "k", "v", "p"):
        wts[nm] = nc.declare_dram_parameter(f"w{nm}t", [C, C], bf16,
                                            isOutput=False)
    dqcol = nc.declare_dram_parameter("dqcol", [P, CCH], f32, isOutput=False)
    bpcol = nc.declare_dram_parameter("bpcol", [P, CCH], f32, isOutput=False)
    maskp = nc.declare_dram_parameter("maskt", [P, 4 * P], bf16, isOutput=False)
    ones_col = nc.declare_dram_parameter("ones_col", [P, 1], bf16, isOutput=False)
    ones_row = nc.declare_dram_parameter("ones_row", [1, C], bf16, isOutput=False)
    outp = nc.declare_dram_parameter("out", [nblk, C, T * W], f32,
                                     isOutput=True)

    with tile.TileContext(nc) as tc:
        with (
            tc.tile_pool(name="const", bufs=1) as const,
            tc.tile_pool(name="xpool", bufs=2) as xpool,
            tc.tile_pool(name="cpool", bufs=2) as cpool,
            tc.tile_pool(name="gpool", bufs=2) as gpool,
            tc.tile_pool(name="spool", bufs=2) as spool,
            tc.tile_pool(name="pp", bufs=3, space="PSUM") as pp,
            tc.tile_pool(name="pss", bufs=2, space="PSUM") as pss,
            tc.tile_pool(name="psm", bufs=1, space="PSUM") as psm,
        ):
            # ---------- constants ----------
            w_sb = {}
            for nm in ("q", "k", "v", "p"):
                for ci in range(CCH):
                    t = const.tile([P, C], bf16, tag=f"w{nm}{ci}")
                    nc.sync.dma_start(t[:], wts[nm][ci * P:(ci + 1) * P, :])
                    w_sb[nm, ci] = t
            dq_sb = const.tile([P, CCH], f32, tag="dqcol")
            nc.sync.dma_start(dq_sb[:], dqcol[:])
            bp_sb = const.tile([P, CCH], f32, tag="bpcol")
            nc.sync.dma_start(bp_sb[:], bpcol[:])
            mask_sb = const.tile([P, 4 * P], bf16, tag="maskt")
            nc.sync.dma_start(mask_sb[:], maskp[:])
            ocr_sb = const.tile([P, 1], bf16, tag="ocr")
            nc.sync.dma_start(ocr_sb[:], ones_col[:])
            orr_sb = const.tile([1, C], bf16, tag="orr")
            nc.sync.dma_start(orr_sb[:], ones_row[:])

            # ---------- main blocks ----------
            for blk in range(nblk):
                xb = []
                xc = []
                for ci in range(CCH):
                    t = xpool.tile([P, T * W], f32r, tag=f"xb{ci}")
                    nc.sync.dma_start(t[:], xs[blk, ci * P:(ci + 1) * P, :])
                    xb.append(t)
                    c = cpool.tile([P, T * W], bf16, tag=f"xc{ci}")
                    nc.gpsimd.tensor_copy(c[:], t[:].bitcast(f32))
                    xc.append(c)

                def xgrp(ci, w0, n=GRP):
                    # [128, n*64] contiguous bf16 slab of group cols (w-major)
                    return xc[ci][:, w0 * T:(w0 + n) * T]

                def xres(ci, w0, n=GRP):
                    # fp32 residual slab
                    return xb[ci][:, w0 * T:(w0 + n) * T]



                for g in range(NGRP):
                    w0 = g * GRP
                    # ---- Q, K projections: psum[co, (w,t)] over ci ----
                    qk = {}
                    for nm in ("q", "k"):
                        for co in range(CCH):
                            ps = pp.tile([P, 512], f32, tag="pp")
                            for ci in range(CCH):
                                nc.tensor.matmul(
                                    ps[:], w_sb[nm, ci][:, co * P:(co + 1) * P],
                                    xgrp(ci, w0), start=(ci == 0),
                                    stop=(ci == CCH - 1))
                            t = gpool.tile([P, 512], bf16, tag=f"{nm}g{co}")
                            if nm == "q":
                                # q += dq at eviction (DVE), cast to bf16
                                nc.vector.tensor_scalar(
                                    t[:], ps[:], dq_sb[:, co:co + 1], None,
                                    ALU.add)
                            else:
                                nc.scalar.copy(t[:], ps[:])
                            qk[nm, co] = t

                    # ---- VT pairs: [128 packed s, 512 co] per pair ----
                    # lhsT = two adjacent locations' x columns [128,128];
                    # rows 0-63 of the result hold the even location's V^T,
                    # rows 64-127 the odd one's
                    vt = []
                    for j in range(GRP // 2):
                        ps = pss.tile([P, 512], f32, tag="ppv")
                        for ci in range(CCH):
                            nc.tensor.matmul(ps[:], xgrp(ci, w0 + 2 * j, 2),
                                             w_sb["v", ci][:],
                                             start=(ci == 0),
                                             stop=(ci == CCH - 1))
                        t = gpool.tile([P, 512], bf16, tag=f"vtg{j}")
                        nc.scalar.copy(t[:], ps[:])
                        vt.append(t)

                    # ---- pair scores: S2[packed s, (pair, t)] ----
                    # one [128,128]x[128,128] matmul per (pair, ci); the
                    # off-diagonal cross-location blocks are garbage and get
                    # zeroed by the block-diagonal causal mask after exp.
                    # one bank holds 4 chains; the first matmul's start=True
                    # clears the whole bank, later chains overwrite where
                    # has_written is unset
                    ps_s = psm.tile([P, 512], f32, tag="pss")
                    for j in range(GRP // 2):
                        for ci in range(CCH):
                            kl = qk["k", ci][:, j * P:(j + 1) * P]
                            ql = qk["q", ci][:, j * P:(j + 1) * P]
                            nc.tensor.matmul(ps_s[:, j * P:(j + 1) * P],
                                             kl, ql,
                                             start=(j == 0 and ci == 0),
                                             stop=(ci == CCH - 1),
                                             skip_group_check=True)
                    # ---- softmax (no max-subtraction) ----
                    pexp = spool.tile([P, 512], bf16, tag="pexp")
                    nc.scalar.activation(pexp[:], ps_s[:], AF.Exp)
                    pm = spool.tile([P, 512], bf16, tag="pmask")
                    nc.vector.tensor_mul(pm[:], pexp[:], mask_sb[:])
                    # masked rows are zero, so full-column sums are exactly
                    # the per-location softmax denominators
                    ps_sum = psm.tile([1, 512], f32, tag="psum_s")
                    nc.tensor.matmul(ps_sum[:], ocr_sb[:], pm[:],
                                     start=True, stop=True)
                    rsf = spool.tile([1, 512], f32, tag="rsf")
                    nc.vector.reciprocal_approx_fast(rsf[:], ps_sum[:])
                    rs = spool.tile([1, 512], bf16, tag="rs")
                    nc.vector.tensor_copy(rs[:], rsf[:])
                    ps_rb = psm.tile([P, 512], f32, tag="psrb")
                    nc.tensor.matmul(ps_rb[:], orr_sb[0:1, 0:P], rs[:],
                                     start=True, stop=True)
                    pn = spool.tile([P, 512], bf16, tag="pn")
                    nc.vector.tensor_mul(pn[:], pm[:], ps_rb[:])

                    # ---- AV: O[c,(w,t)], one matmul per (pair, c-chunk)
                    # contracting over all 128 packed s rows (zeroed entries
                    # of pn contribute nothing) ----
                    og = []
                    for ch in range(CCH):
                        ps_o = pp.tile([P, 512], f32, tag="pp")
                        for j in range(GRP // 2):
                            lhsT = vt[j][:, ch * P:(ch + 1) * P]
                            nc.tensor.matmul(ps_o[:, j * P:(j + 1) * P],
                                             lhsT, pn[:, j * P:(j + 1) * P],
                                             start=(j == 0), stop=True,
                                             skip_group_check=True)
                        t = gpool.tile([P, 512], bf16, tag=f"og{ch}")
                        nc.scalar.copy(t[:], ps_o[:])
                        og.append(t)

                    # ---- P-projection + bias + residual ----
                    for co in range(CCH):
                        ps = pp.tile([P, 512], f32, tag="pp")
                        for ci in range(CCH):
                            nc.tensor.matmul(
                                ps[:], w_sb["p", ci][:, co * P:(co + 1) * P],
                                og[ci][:], start=(ci == 0),
                                stop=(ci == CCH - 1))
                        # x += (ps + bp_eff): fused bias + residual on DVE
                        xsl = xres(co, w0)
                        nc.vector.scalar_tensor_tensor(
                            xsl, ps[:], bp_sb[:, co:co + 1], xsl.bitcast(f32),
                            ALU.add, ALU.add)

                for ci in range(CCH):
                    nc.sync.dma_start(outp[blk, ci * P:(ci + 1) * P, :],
                                      xb[ci][:].bitcast(f32))
    nc.compile()
    return nc


def host_prep(x, gamma, beta, wq, bq, wk, bk, wv, bv, wp, bp):
    """Per-batch GroupNorm stats + fold gamma/beta/mean/rstd into weights.

    Returns a list of per-batch constant dicts (cores 0-3 use batch 0,
    cores 4-7 use batch 1)."""
    n = C * T * H * W
    s = np.float32(1.0 / np.sqrt(C))
    g64 = gamma.astype(np.float64)
    b64 = beta.astype(np.float64)

    # block-diagonal causal mask for packed location pairs: [[M,0],[0,M]]
    # per pair (M = triu), tiled across the 4 pairs of a group
    m1 = np.triu(np.ones((T, T), np.float32))
    z = np.zeros((T, T), np.float32)
    mblk = np.block([[m1, z], [z, m1]])                       # (128, 128)
    maskt = np.tile(mblk, (1, 4))                             # (128, 512)
    shared = {
        "maskt": maskt.astype(ml_dtypes.bfloat16),
        "ones_col": np.ones((P, 1), ml_dtypes.bfloat16),
        "ones_row": np.ones((1, C), ml_dtypes.bfloat16),
    }

    out = []
    for b in range(B):
        y = x[b].reshape(-1)
        s1 = float(np.add.reduce(y, dtype=np.float64))
        s2 = float(np.add.reduce(np.square(y, dtype=np.float64)))
        mu = s1 / n
        var = s2 / n - mu * mu
        r = 1.0 / np.sqrt(var + EPS)

        def fold(w, bias, scale):
            w64 = w.astype(np.float64)
            a = (w64 * g64[None, :]) * (scale * r)            # (co, ci)
            d = (bias.astype(np.float64) + w64 @ b64
                 - (mu * r) * (w64 @ g64)) * scale            # (co,)
            return np.ascontiguousarray(a.T.astype(ml_dtypes.bfloat16)), d

        aqt, dq = fold(wq, bq, s)
        akt, _ = fold(wk, bk, 1.0)
        avt, dv = fold(wv, bv, 1.0)
        apt = np.ascontiguousarray(wp.T.astype(ml_dtypes.bfloat16))
        bp_eff = bp.astype(np.float64) + wp.astype(np.float64) @ dv

        dqcol = np.empty((P, CCH), np.float32)
        bpcol = np.empty((P, CCH), np.float32)
        for ch in range(CCH):
            dqcol[:, ch] = dq[ch * P:(ch + 1) * P]
            bpcol[:, ch] = bp_eff[ch * P:(ch + 1) * P]

        out.append({
            "wqt": aqt, "wkt": akt, "wvt": avt, "wpt": apt,
            "dqcol": dqcol, "bpcol": bpcol, **shared,
        })
    return out


_NC_CACHE = {}


def kernel(x, gamma, beta, wq, bq, wk, bk, wv, bv, wp, bp):
    x = np.asarray(x, np.float32)
    args = [np.asarray(a, np.float32) for a in
            (gamma, beta, wq, bq, wk, bk, wv, bv, wp, bp)]
    consts = host_prep(x, *args)

    if "nc" not in _NC_CACHE:
        _NC_CACHE["nc"] = build_nc()
    nc = _NC_CACHE["nc"]

    in_maps = []
    for core in range(NCORES):
        b, hg = core // 4, core % 4
        shard = x[b, :, :, hg * HSH:(hg + 1) * HSH, :]        # (C,T,HSH,W)
        shard = np.ascontiguousarray(
            shard.transpose(2, 0, 3, 1)).reshape(HSH, C, W * T)
        in_maps.append({"xs": shard, **consts[b]})

    global _last_in_maps
    _last_in_maps = in_maps
    res = run_bass_kernel_spmd(nc, in_maps, list(range(NCORES)))

    out = np.empty((B, C, T, H, W), np.float32)
    for core in range(NCORES):
        b, hg = core // 4, core % 4
        o = res.results[core]["out"].reshape(HSH, C, W, T)
        out[b, :, :, hg * HSH:(hg + 1) * HSH, :] = o.transpose(1, 3, 0, 2)
    return out
